# revision 32
# baseline (speedup 1.0000x reference)
"""Llama4 MoE experts + shared LoRA, expert-parallel on 8 TRN2 NeuronCores.

Per-core (expert e): x[1024,1024] @ W_gu[1024,4096] (+ rank-8 LoRA) -> SwiGLU
-> h[1024,2048] @ W_dn[2048,1024] (+ rank-8 LoRA) -> out[1024,1024].

All matmul operands are bf16 (native 1 cycle/column on TRN2; fp32/f32r pay a
hi/lo split, measured ~2x). Key structure:
- x is cast to bf16 (DVE) and transposed by the DMA XBAR, never by the PE;
  the x chain runs on the scalar engine's HWDGE queue, weights on sync.
- W_gu streams in quad-chunks [128, 8, 512] so DMA descriptors are 2 KB
  (512 B descriptors throttle the DMA stream below PE rate).
- B_gu loads/transposes are split and interleaved per-iteration; its strided
  32 B-descriptor DMA is chopped into quads between W_gu quads.
- W_dn rows are prefetched + cast to bf16 during the back half of phase B and
  stay resident (32 KB/partition): phase D is a pure matmul stream.
- Matmuls sharing a stationary operand are issued back-to-back (both T-chunks
  per W_gu tile, both H-halves per hT chunk).
PSUM accumulates in f32; output is stored f32.
"""
import sys

sys.path.insert(0, "/opt/trn_rl_repo")

import numpy as np

import concourse.bacc as bacc
import concourse.bass as bass
import concourse.mybir as mybir
import concourse.tile as tile
from concourse.bass_utils import run_bass_kernel_spmd
from concourse.masks import make_identity

E = 8           # experts == cores
T = 1024        # tokens per expert
H = 1024        # hidden
I = 2048        # intermediate
F2 = 2 * I      # gate+up
R = 8           # lora rank
SCALING = 2.0   # lora_alpha / rank
P = 128         # partitions
NFREE = 512     # moving free-dim per matmul (one PSUM bank of fp32)
KH = H // P     # 8 k-tiles over H
KI = I // P     # 16 k-tiles over I
NT = T // NFREE     # 2 T-chunks
NH = H // NFREE     # 2 H-chunks
NFP = I // P        # 16 F-pair tiles (gate i pairs with up i+16)
QW = 4 * P          # 512-wide W_gu quad chunks

F32 = mybir.dt.float32
BF16 = mybir.dt.bfloat16


def build_kernel():
    nc = bacc.Bacc("TRN2", target_bir_lowering=False, debug=False)

    x_d = nc.dram_tensor("x", [T, H], F32, kind="ExternalInput")
    wgu_d = nc.dram_tensor("w_gu", [H, F2], F32, kind="ExternalInput")
    wdn_d = nc.dram_tensor("w_dn", [I, H], F32, kind="ExternalInput")
    agu_d = nc.dram_tensor("a_gu", [R, H], F32, kind="ExternalInput")
    bgu_d = nc.dram_tensor("b_gu", [F2, R], F32, kind="ExternalInput")
    adn_d = nc.dram_tensor("a_dn", [R, I], F32, kind="ExternalInput")
    bdn_d = nc.dram_tensor("b_dn", [H, R], F32, kind="ExternalInput")
    out_d = nc.dram_tensor("out", [T, H], F32, kind="ExternalOutput")

    with tile.TileContext(nc) as tc:
        with (
            tc.tile_pool(name="const", bufs=1) as const_pool,
            tc.tile_pool(name="xT", bufs=1) as xT_pool,
            tc.tile_pool(name="hT", bufs=1) as hT_pool,
            tc.tile_pool(name="wdnb", bufs=1) as wdnb_pool,
            tc.tile_pool(name="smalls", bufs=1) as small_pool,
            tc.tile_pool(name="xnat", bufs=2) as xnat_pool,
            tc.tile_pool(name="xbf", bufs=2) as xbf_pool,
            tc.tile_pool(name="wgu", bufs=4) as wgu_pool,
            tc.tile_pool(name="wgub", bufs=4) as wgub_pool,
            tc.tile_pool(name="wdn", bufs=5) as wdn_pool,
            tc.tile_pool(name="silu", bufs=2) as silu_pool,
            tc.tile_pool(name="outs", bufs=2) as out_pool,
            tc.tile_pool(name="ps_tr", bufs=2, space="PSUM") as ps_tr,
            tc.tile_pool(name="ps_mm", bufs=6, space="PSUM") as ps_mm,
        ):
            ident = const_pool.tile([P, P], F32)
            make_identity(nc, ident[:])

            # ---- lora A DMAs first on sync (tiny, contiguous: PE's first work)
            agu_nat = small_pool.tile([R, H], F32, tag="agu_nat")
            nc.sync.dma_start(agu_nat[:], agu_d[:])
            adn_nat = small_pool.tile([R, I], F32, tag="adn_nat")
            nc.sync.dma_start(adn_nat[:], adn_d[:])

            # ---- phase A: x -> bf16 -> XBAR transpose -> xT[p, k, t] ----
            # xT[:, k, t] = x[t, 128k + p]; the whole x chain rides the scalar
            # HWDGE queue so sync is pure W_gu from t=0. B_gu pair DMAs slot
            # into the scalar queue right behind the x halves they follow.
            bgu_nat = small_pool.tile([P, F2 // P, R], F32, tag="bgu_nat")
            bguT = small_pool.tile([R, F2], BF16, tag="bguT")
            bdn_nat = small_pool.tile([P, H // P, R], F32, tag="bdn_nat")
            bdnT = small_pool.tile([R, H], BF16, tag="bdnT")

            def fetch_bgu_pairs(iters):
                for i in iters:
                    for bo in (i, NFP + i):
                        nc.scalar.dma_start(
                            bgu_nat[:, bo, :],
                            bgu_d[P * bo:P * (bo + 1), :].rearrange("(bo bi) r -> bi bo r", bi=P))

            xT = xT_pool.tile([P, KH, T], BF16, tag="xT", name="xT")
            for b in range(T // P):
                xb = xnat_pool.tile([P, H], F32)
                nc.scalar.dma_start(xb[:], x_d[P * b:P * (b + 1), :])
                xbf = xbf_pool.tile([P, H], BF16)
                nc.vector.tensor_copy(xbf[:], xb[:])
                nc.scalar.dma_start(xT[:, :, P * b:P * (b + 1)], xbf[:],
                                    transpose=True)
                if b == 3:
                    fetch_bgu_pairs(range(0, 4))
            fetch_bgu_pairs(range(4, NFP))

            # ---- A_guT[k]: [128 H, 8 R] (first PE work, in the DMA shadow)
            aguT = [small_pool.tile([P, R], BF16, tag=f"aguT{k}", name=f"aguT{k}") for k in range(KH)]
            for k in range(KH):
                ps = ps_tr.tile([P, R], F32, tag="tr")
                nc.tensor.transpose(ps[:], agu_nat[:, P * k:P * (k + 1)],
                                    ident[:R, :R])
                nc.vector.tensor_copy(aguT[k][:], ps[:])
            # ---- A_dnT[k]: [128 I, 8 R]
            adnT = [small_pool.tile([P, R], BF16, tag=f"adnT{k}", name=f"adnT{k}") for k in range(KI)]
            for k in range(KI):
                ps = ps_tr.tile([P, R], F32, tag="tr")
                nc.tensor.transpose(ps[:], adn_nat[:, P * k:P * (k + 1)],
                                    ident[:R, :R])
                nc.vector.tensor_copy(adnT[k][:], ps[:])

            # ---- r1T = SCALING * (A_gu @ x^T): [8 R, 1024 T] bf16 ----
            # t-major: the t=0 half only needs x blocks 0-3, so the PE starts
            # ~8 us before the full xT transpose completes. r1's t=1 half is
            # emitted inside phase B's first iteration (after its t=0 block)
            # so it never stalls the t=0 stream.
            r1T = small_pool.tile([R, T], BF16, tag="r1T")

            def emit_r1(t):
                rps = ps_mm.tile([R, NFREE], F32, tag="mm", name=f"r1ps{t}")
                for k in range(KH):
                    nc.tensor.matmul(rps[:], aguT[k][:],
                                     xT[:, k, NFREE * t:NFREE * (t + 1)],
                                     start=(k == 0), stop=(k == KH - 1))
                nc.vector.tensor_scalar_mul(
                    r1T[:, NFREE * t:NFREE * (t + 1)], rps[:], SCALING)

            emit_r1(0)

            # ---- phase B: gate_up^T + SwiGLU -> hiddenT[k] [128 I, 1024 T] ----
            hT = [hT_pool.tile([P, T], BF16, tag=f"hT{k}", name=f"hT{k}") for k in range(KI)]
            wdnb = [wdnb_pool.tile([P, H], BF16, tag=f"wdnb{k}", name=f"wdnb{k}")
                    for k in range(KI)]
            wdn_stage = {}
            wgu_stage = {}
            wgub_tiles = {}

            # Queue plan: sync carries iters 0-11 of W_gu (12 MB) and nothing
            # else until the output writes; the scalar queue (free after the x
            # chain) carries W_dn, then iters 12-15 of W_gu, then B_dn. This
            # keeps the W_gu stream just ahead of the PE on both ends.
            def iter_head(i):
                fg, fu = P * i, P * (i + NFP)
                if i < 12:
                    wg = wgu_pool.tile([P, KH, P], F32, tag="wgu", name=f"wg{i}")
                    wu = wgu_pool.tile([P, KH, P], F32, tag="wgu", name=f"wu{i}")
                    nc.sync.dma_start(
                        wg[:], wgu_d[:, fg:fg + P].rearrange("(ko ki) f -> ki ko f", ki=P))
                    nc.sync.dma_start(
                        wu[:], wgu_d[:, fu:fu + P].rearrange("(ko ki) f -> ki ko f", ki=P))
                else:
                    wg, wu = wgu_stage.pop(i)
                # W_dn rows on scalar, right behind the x chain
                if i < 8:
                    for k in (2 * i, 2 * i + 1):
                        wd = wdn_pool.tile([P, H], F32, tag="wdn")
                        nc.scalar.dma_start(wd[:], wdn_d[P * k:P * (k + 1), :])
                        wdn_stage[k] = wd
                if 2 <= i < 10:
                    for k in (2 * (i - 2), 2 * (i - 2) + 1):
                        nc.vector.tensor_copy(wdnb[k][:], wdn_stage.pop(k)[:])
                # late W_gu quarter staged via the scalar queue
                if 8 <= i < 12:
                    j = i + 4
                    fgj, fuj = P * j, P * (j + NFP)
                    wgL = wgu_pool.tile([P, KH, P], F32, tag="wguL", bufs=8, name=f"wgL{j}")
                    wuL = wgu_pool.tile([P, KH, P], F32, tag="wguL", bufs=8, name=f"wuL{j}")
                    nc.scalar.dma_start(
                        wgL[:], wgu_d[:, fgj:fgj + P].rearrange("(ko ki) f -> ki ko f", ki=P))
                    nc.scalar.dma_start(
                        wuL[:], wgu_d[:, fuj:fuj + P].rearrange("(ko ki) f -> ki ko f", ki=P))
                    wgu_stage[j] = (wgL, wuL)
                if i == 11:
                    nc.scalar.dma_start(
                        bdn_nat[:],
                        bdn_d[:].rearrange("(bo bi) r -> bi bo r", bi=P))
                wgb = wgub_pool.tile([P, KH, P], BF16, tag="wgub", name=f"wgb{i}")
                wub = wgub_pool.tile([P, KH, P], BF16, tag="wgub", name=f"wub{i}")
                nc.vector.tensor_copy(wgb[:], wg[:])
                nc.gpsimd.tensor_copy(wub[:], wu[:])
                # B_guT columns for this iteration (PE transpose + copy)
                for bo in (i, NFP + i):
                    ps = ps_tr.tile([R, P], F32, tag="tr")
                    nc.tensor.transpose(ps[:], bgu_nat[:, bo, :], ident[:])
                    nc.vector.tensor_copy(bguT[:, P * bo:P * (bo + 1)], ps[:])
                wgub_tiles[i] = (wgb, wub)

            def iter_block(i, t):
                fg, fu = P * i, P * (i + NFP)
                wgb, wub = wgub_tiles[i]
                ts = slice(NFREE * t, NFREE * (t + 1))
                psg = ps_mm.tile([P, NFREE], F32, tag="mm", name=f"psg{i}_{t}")
                psu = ps_mm.tile([P, NFREE], F32, tag="mm", name=f"psu{i}_{t}")
                for k in range(KH):
                    nc.tensor.matmul(psg[:], wgb[:, k, :],
                                     xT[:, k, ts], start=(k == 0), stop=False)
                nc.tensor.matmul(psg[:], bguT[:, fg:fg + P], r1T[:, ts],
                                 start=False, stop=True)
                for k in range(KH):
                    nc.tensor.matmul(psu[:], wub[:, k, :],
                                     xT[:, k, ts], start=(k == 0), stop=False)
                nc.tensor.matmul(psu[:], bguT[:, fu:fu + P], r1T[:, ts],
                                 start=False, stop=True)
                sg = silu_pool.tile([P, NFREE], F32, tag="silu")
                nc.scalar.activation(sg[:], psg[:],
                                     mybir.ActivationFunctionType.Silu)
                nc.vector.tensor_mul(hT[i][:, ts], sg[:], psu[:])

            # iters 0-1 run t=0 first so the PE never waits on xT's second
            # half (x blocks 4-7), with r1's t=1 pass slotted in between.
            iter_head(0)
            iter_block(0, 0)
            iter_head(1)
            iter_block(1, 0)
            emit_r1(1)
            iter_block(0, 1)
            iter_block(1, 1)
            for i in range(2, NFP):
                iter_head(i)
                iter_block(i, 0)
                iter_block(i, 1)

            # ---- B_dnT: [8 R, 1024 H] (PE work hidden at the phase boundary)
            for bo in range(H // P):
                ps = ps_tr.tile([R, P], F32, tag="tr")
                nc.tensor.transpose(ps[:], bdn_nat[:, bo, :], ident[:])
                nc.vector.tensor_copy(bdnT[:, P * bo:P * (bo + 1)], ps[:])

            # ---- r2T = SCALING * (A_dn @ hidden^T): [8 R, 1024 T] bf16 ----
            r2T = small_pool.tile([R, T], BF16, tag="r2T")
            rps2 = [ps_mm.tile([R, NFREE], F32, tag="mm", name=f"r2ps{t}")
                    for t in range(NT)]
            for k in range(KI):
                for t in range(NT):
                    nc.tensor.matmul(rps2[t][:], adnT[k][:],
                                     hT[k][:, NFREE * t:NFREE * (t + 1)],
                                     start=(k == 0), stop=(k == KI - 1))
            for t in range(NT):
                nc.vector.tensor_scalar_mul(
                    r2T[:, NFREE * t:NFREE * (t + 1)], rps2[t][:], SCALING)

            # ---- phase D: out[T, H] = hidden @ W_dn + lora ----
            # wdnb[k] resident bf16: pure matmul stream, no DMA/cast here.
            # Per stationary hT chunk, both H-halves issue back-to-back.
            nout = 0
            for grp in range(4):
                pos = [[ps_mm.tile([P, NFREE], F32, tag="mm",
                                   name=f"po{grp}_{jj}_{h}")
                        for h in range(NH)] for jj in range(2)]
                for k in range(KI):
                    for jj in range(2):
                        j = 2 * grp + jj
                        for h in range(NH):
                            hs = slice(NFREE * h, NFREE * (h + 1))
                            nc.tensor.matmul(pos[jj][h][:],
                                             hT[k][:, P * j:P * (j + 1)],
                                             wdnb[k][:, hs],
                                             start=(k == 0), stop=False)
                for jj in range(2):
                    j = 2 * grp + jj
                    for h in range(NH):
                        hs = slice(NFREE * h, NFREE * (h + 1))
                        nc.tensor.matmul(pos[jj][h][:], r2T[:, P * j:P * (j + 1)],
                                         bdnT[:, hs], start=False, stop=True)
                        ot = out_pool.tile([P, NFREE], F32, tag="outs")
                        if nout % 2 == 0:
                            nc.scalar.activation(ot[:], pos[jj][h][:],
                                                 mybir.ActivationFunctionType.Copy)
                        else:
                            nc.vector.tensor_copy(ot[:], pos[jj][h][:])
                        nout += 1
                        nc.sync.dma_start(out_d[P * j:P * (j + 1), hs], ot[:])

    nc.finalize()
    return nc


_NC_CACHE = None


def _get_nc():
    global _NC_CACHE
    if _NC_CACHE is None:
        _NC_CACHE = build_kernel()
    return _NC_CACHE


def _run(hidden_states, gate_up_proj, down_proj,
         lora_A_gu, lora_B_gu, lora_A_dn, lora_B_dn, **spmd_kwargs):
    f32 = np.float32
    hidden_states = np.ascontiguousarray(hidden_states, dtype=f32)
    gate_up_proj = np.ascontiguousarray(gate_up_proj, dtype=f32)
    down_proj = np.ascontiguousarray(down_proj, dtype=f32)
    lora_A_gu = np.ascontiguousarray(lora_A_gu, dtype=f32)
    lora_B_gu = np.ascontiguousarray(lora_B_gu, dtype=f32)
    lora_A_dn = np.ascontiguousarray(lora_A_dn, dtype=f32)
    lora_B_dn = np.ascontiguousarray(lora_B_dn, dtype=f32)

    nc = _get_nc()
    in_maps = []
    for e in range(E):
        in_maps.append({
            "x": hidden_states[T * e:T * (e + 1), :],
            "w_gu": gate_up_proj[e],
            "w_dn": down_proj[e],
            "a_gu": lora_A_gu,
            "b_gu": lora_B_gu,
            "a_dn": lora_A_dn,
            "b_dn": lora_B_dn,
        })
    res = run_bass_kernel_spmd(nc, in_maps, core_ids=list(range(E)),
                               **spmd_kwargs)
    out = np.concatenate([res.results[e]["out"] for e in range(E)], axis=0)
    return out, res


def kernel(hidden_states, gate_up_proj, down_proj,
           lora_A_gu, lora_B_gu, lora_A_dn, lora_B_dn):
    out, _ = _run(hidden_states, gate_up_proj, down_proj,
                  lora_A_gu, lora_B_gu, lora_A_dn, lora_B_dn)
    return out


# revision 35
# speedup vs baseline: 1.0344x; 1.0344x over previous
"""Llama4 MoE experts + shared LoRA, expert-parallel on 8 TRN2 NeuronCores.

Per-core (expert e): x[1024,1024] @ W_gu[1024,4096] (+ rank-8 LoRA) -> SwiGLU
-> h[1024,2048] @ W_dn[2048,1024] (+ rank-8 LoRA) -> out[1024,1024].

All matmul operands are bf16 (native 1 cycle/column on TRN2; fp32/f32r pay a
hi/lo split, measured ~2x). Key structure:
- x is cast to bf16 (DVE) and transposed by the DMA XBAR, never by the PE;
  the x chain runs on the scalar engine's HWDGE queue, weights on sync.
- W_gu streams in quad-chunks [128, 8, 512] so DMA descriptors are 2 KB
  (512 B descriptors throttle the DMA stream below PE rate).
- B_gu loads/transposes are split and interleaved per-iteration; its strided
  32 B-descriptor DMA is chopped into quads between W_gu quads.
- W_dn rows are prefetched + cast to bf16 during the back half of phase B and
  stay resident (32 KB/partition): phase D is a pure matmul stream.
- Matmuls sharing a stationary operand are issued back-to-back (both T-chunks
  per W_gu tile, both H-halves per hT chunk).
PSUM accumulates in f32; output is stored f32.
"""
import sys

sys.path.insert(0, "/opt/trn_rl_repo")

import numpy as np

import concourse.bacc as bacc
import concourse.bass as bass
import concourse.mybir as mybir
import concourse.tile as tile
from concourse.bass_utils import run_bass_kernel_spmd
from concourse.masks import make_identity

E = 8           # experts == cores
T = 1024        # tokens per expert
H = 1024        # hidden
I = 2048        # intermediate
F2 = 2 * I      # gate+up
R = 8           # lora rank
SCALING = 2.0   # lora_alpha / rank
P = 128         # partitions
NFREE = 512     # moving free-dim per matmul (one PSUM bank of fp32)
KH = H // P     # 8 k-tiles over H
KI = I // P     # 16 k-tiles over I
NT = T // NFREE     # 2 T-chunks
NH = H // NFREE     # 2 H-chunks
NFP = I // P        # 16 F-pair tiles (gate i pairs with up i+16)
QW = 4 * P          # 512-wide W_gu quad chunks

F32 = mybir.dt.float32
BF16 = mybir.dt.bfloat16


def build_kernel():
    nc = bacc.Bacc("TRN2", target_bir_lowering=False, debug=False)

    x_d = nc.dram_tensor("x", [T, H], F32, kind="ExternalInput")
    wgu_d = nc.dram_tensor("w_gu", [H, F2], F32, kind="ExternalInput")
    wdn_d = nc.dram_tensor("w_dn", [I, H], F32, kind="ExternalInput")
    agu_d = nc.dram_tensor("a_gu", [R, H], F32, kind="ExternalInput")
    bgu_d = nc.dram_tensor("b_gu", [F2, R], F32, kind="ExternalInput")
    adn_d = nc.dram_tensor("a_dn", [R, I], F32, kind="ExternalInput")
    bdn_d = nc.dram_tensor("b_dn", [H, R], F32, kind="ExternalInput")
    out_d = nc.dram_tensor("out", [T, H], F32, kind="ExternalOutput")

    with tile.TileContext(nc) as tc:
        with (
            tc.tile_pool(name="const", bufs=1) as const_pool,
            tc.tile_pool(name="xT", bufs=1) as xT_pool,
            tc.tile_pool(name="hT", bufs=1) as hT_pool,
            tc.tile_pool(name="wdnb", bufs=1) as wdnb_pool,
            tc.tile_pool(name="smalls", bufs=1) as small_pool,
            tc.tile_pool(name="xnat", bufs=2) as xnat_pool,
            tc.tile_pool(name="xbf", bufs=2) as xbf_pool,
            tc.tile_pool(name="wgu", bufs=4) as wgu_pool,
            tc.tile_pool(name="wgub", bufs=4) as wgub_pool,
            tc.tile_pool(name="wdn", bufs=5) as wdn_pool,
            tc.tile_pool(name="silu", bufs=2) as silu_pool,
            tc.tile_pool(name="outs", bufs=2) as out_pool,
            tc.tile_pool(name="ps_tr", bufs=2, space="PSUM") as ps_tr,
            tc.tile_pool(name="ps_mm", bufs=6, space="PSUM") as ps_mm,
        ):
            ident = const_pool.tile([P, P], F32)
            make_identity(nc, ident[:])

            # ---- lora A DMAs first on sync (tiny, contiguous: PE's first work)
            agu_nat = small_pool.tile([R, H], F32, tag="agu_nat")
            nc.sync.dma_start(agu_nat[:], agu_d[:])
            adn_nat = small_pool.tile([R, I], F32, tag="adn_nat")
            nc.sync.dma_start(adn_nat[:], adn_d[:])

            # ---- phase A: x -> bf16 -> XBAR transpose -> xT[p, k, t] ----
            # xT[:, k, t] = x[t, 128k + p]; the whole x chain rides the scalar
            # HWDGE queue so sync is pure W_gu from t=0. B_gu pair DMAs slot
            # into the scalar queue right behind the x halves they follow.
            bgu_nat = small_pool.tile([P, F2 // P, R], F32, tag="bgu_nat")
            bguT = small_pool.tile([R, F2], BF16, tag="bguT")
            bdn_nat = small_pool.tile([P, H // P, R], F32, tag="bdn_nat")
            bdnT = small_pool.tile([R, H], BF16, tag="bdnT")

            def fetch_bgu_pairs(iters):
                for i in iters:
                    for bo in (i, NFP + i):
                        nc.scalar.dma_start(
                            bgu_nat[:, bo, :],
                            bgu_d[P * bo:P * (bo + 1), :].rearrange("(bo bi) r -> bi bo r", bi=P))

            xT = xT_pool.tile([P, KH, T], BF16, tag="xT", name="xT")
            for b in range(T // P):
                xb = xnat_pool.tile([P, H], F32)
                nc.scalar.dma_start(xb[:], x_d[P * b:P * (b + 1), :])
                xbf = xbf_pool.tile([P, H], BF16)
                nc.vector.tensor_copy(xbf[:], xb[:])
                nc.scalar.dma_start(xT[:, :, P * b:P * (b + 1)], xbf[:],
                                    transpose=True)

            # ---- A_guT[k]: [128 H, 8 R] (first PE work, in the DMA shadow)
            aguT = [small_pool.tile([P, R], BF16, tag=f"aguT{k}", name=f"aguT{k}") for k in range(KH)]
            for k in range(KH):
                ps = ps_tr.tile([P, R], F32, tag="tr")
                nc.tensor.transpose(ps[:], agu_nat[:, P * k:P * (k + 1)],
                                    ident[:R, :R])
                nc.vector.tensor_copy(aguT[k][:], ps[:])
            # ---- A_dnT[k]: [128 I, 8 R]
            adnT = [small_pool.tile([P, R], BF16, tag=f"adnT{k}", name=f"adnT{k}") for k in range(KI)]
            for k in range(KI):
                ps = ps_tr.tile([P, R], F32, tag="tr")
                nc.tensor.transpose(ps[:], adn_nat[:, P * k:P * (k + 1)],
                                    ident[:R, :R])
                nc.vector.tensor_copy(adnT[k][:], ps[:])

            # ---- r1T = SCALING * (A_gu @ x^T): [8 R, 1024 T] bf16 ----
            # t-major: the t=0 half only needs x blocks 0-3, so the PE starts
            # ~8 us before the full xT transpose completes. r1's t=1 half is
            # emitted inside phase B's first iteration (after its t=0 block)
            # so it never stalls the t=0 stream.
            r1T = small_pool.tile([R, T], BF16, tag="r1T")

            def emit_r1(t):
                rps = ps_mm.tile([R, NFREE], F32, tag="mm", name=f"r1ps{t}")
                for k in range(KH):
                    nc.tensor.matmul(rps[:], aguT[k][:],
                                     xT[:, k, NFREE * t:NFREE * (t + 1)],
                                     start=(k == 0), stop=(k == KH - 1))
                nc.vector.tensor_scalar_mul(
                    r1T[:, NFREE * t:NFREE * (t + 1)], rps[:], SCALING)

            emit_r1(0)

            # ---- phase B: gate_up^T + SwiGLU -> hiddenT[k] [128 I, 1024 T] ----
            hT = [hT_pool.tile([P, T], BF16, tag=f"hT{k}", name=f"hT{k}") for k in range(KI)]
            wdnb = [wdnb_pool.tile([P, H], BF16, tag=f"wdnb{k}", name=f"wdnb{k}")
                    for k in range(KI)]
            wdn_stage = {}
            wgu_stage = {}
            wgub_tiles = {}

            # Queue plan: sync carries iters 0-11 of W_gu (12 MB) and nothing
            # else until the output writes; the scalar queue (free after the x
            # chain) carries W_dn, then iters 12-15 of W_gu, then B_dn. This
            # keeps the W_gu stream just ahead of the PE on both ends.
            def iter_head(i):
                fg, fu = P * i, P * (i + NFP)
                if i < 12:
                    wg = wgu_pool.tile([P, KH, P], F32, tag="wgu", name=f"wg{i}")
                    wu = wgu_pool.tile([P, KH, P], F32, tag="wgu", name=f"wu{i}")
                    nc.sync.dma_start(
                        wg[:], wgu_d[:, fg:fg + P].rearrange("(ko ki) f -> ki ko f", ki=P))
                    nc.sync.dma_start(
                        wu[:], wgu_d[:, fu:fu + P].rearrange("(ko ki) f -> ki ko f", ki=P))
                else:
                    wg, wu = wgu_stage.pop(i)
                # B_gu pair for this iter: 32 B-packet DMAs are cheap on sync
                # (engine-shared with the W stream) but poison the x chain
                for bo in (i, NFP + i):
                    nc.sync.dma_start(
                        bgu_nat[:, bo, :],
                        bgu_d[P * bo:P * (bo + 1), :].rearrange("(bo bi) r -> bi bo r", bi=P))
                # W_dn rows on scalar, right behind the x chain
                if i < 8:
                    for k in (2 * i, 2 * i + 1):
                        wd = wdn_pool.tile([P, H], F32, tag="wdn")
                        nc.scalar.dma_start(wd[:], wdn_d[P * k:P * (k + 1), :])
                        wdn_stage[k] = wd
                if 2 <= i < 10:
                    for k in (2 * (i - 2), 2 * (i - 2) + 1):
                        nc.vector.tensor_copy(wdnb[k][:], wdn_stage.pop(k)[:])
                # late W_gu quarter staged via the scalar queue
                if 6 <= i < 10:
                    j = i + 6
                    fgj, fuj = P * j, P * (j + NFP)
                    wgL = wgu_pool.tile([P, KH, P], F32, tag="wguL", bufs=8, name=f"wgL{j}")
                    wuL = wgu_pool.tile([P, KH, P], F32, tag="wguL", bufs=8, name=f"wuL{j}")
                    nc.scalar.dma_start(
                        wgL[:], wgu_d[:, fgj:fgj + P].rearrange("(ko ki) f -> ki ko f", ki=P))
                    nc.scalar.dma_start(
                        wuL[:], wgu_d[:, fuj:fuj + P].rearrange("(ko ki) f -> ki ko f", ki=P))
                    wgu_stage[j] = (wgL, wuL)
                if i == 11:
                    nc.scalar.dma_start(
                        bdn_nat[:],
                        bdn_d[:].rearrange("(bo bi) r -> bi bo r", bi=P))
                wgb = wgub_pool.tile([P, KH, P], BF16, tag="wgub", name=f"wgb{i}")
                wub = wgub_pool.tile([P, KH, P], BF16, tag="wgub", name=f"wub{i}")
                nc.vector.tensor_copy(wgb[:], wg[:])
                nc.gpsimd.tensor_copy(wub[:], wu[:])
                # B_guT columns for this iteration (PE transpose + copy)
                for bo in (i, NFP + i):
                    ps = ps_tr.tile([R, P], F32, tag="tr")
                    nc.tensor.transpose(ps[:], bgu_nat[:, bo, :], ident[:])
                    nc.vector.tensor_copy(bguT[:, P * bo:P * (bo + 1)], ps[:])
                wgub_tiles[i] = (wgb, wub)

            def iter_block(i, t):
                fg, fu = P * i, P * (i + NFP)
                wgb, wub = wgub_tiles[i]
                ts = slice(NFREE * t, NFREE * (t + 1))
                psg = ps_mm.tile([P, NFREE], F32, tag="mm", name=f"psg{i}_{t}")
                psu = ps_mm.tile([P, NFREE], F32, tag="mm", name=f"psu{i}_{t}")
                for k in range(KH):
                    nc.tensor.matmul(psg[:], wgb[:, k, :],
                                     xT[:, k, ts], start=(k == 0), stop=False)
                nc.tensor.matmul(psg[:], bguT[:, fg:fg + P], r1T[:, ts],
                                 start=False, stop=True)
                for k in range(KH):
                    nc.tensor.matmul(psu[:], wub[:, k, :],
                                     xT[:, k, ts], start=(k == 0), stop=False)
                nc.tensor.matmul(psu[:], bguT[:, fu:fu + P], r1T[:, ts],
                                 start=False, stop=True)
                sg = silu_pool.tile([P, NFREE], F32, tag="silu")
                nc.scalar.activation(sg[:], psg[:],
                                     mybir.ActivationFunctionType.Silu)
                nc.vector.tensor_mul(hT[i][:, ts], sg[:], psu[:])

            # iters 0-1 run t=0 first so the PE never waits on xT's second
            # half (x blocks 4-7), with r1's t=1 pass slotted in between.
            iter_head(0)
            iter_block(0, 0)
            iter_head(1)
            iter_block(1, 0)
            emit_r1(1)
            iter_block(0, 1)
            iter_block(1, 1)
            for i in range(2, NFP):
                iter_head(i)
                iter_block(i, 0)
                iter_block(i, 1)

            # ---- B_dnT: [8 R, 1024 H] (PE work hidden at the phase boundary)
            for bo in range(H // P):
                ps = ps_tr.tile([R, P], F32, tag="tr")
                nc.tensor.transpose(ps[:], bdn_nat[:, bo, :], ident[:])
                nc.vector.tensor_copy(bdnT[:, P * bo:P * (bo + 1)], ps[:])

            # ---- r2T = SCALING * (A_dn @ hidden^T): [8 R, 1024 T] bf16 ----
            r2T = small_pool.tile([R, T], BF16, tag="r2T")
            rps2 = [ps_mm.tile([R, NFREE], F32, tag="mm", name=f"r2ps{t}")
                    for t in range(NT)]
            for k in range(KI):
                for t in range(NT):
                    nc.tensor.matmul(rps2[t][:], adnT[k][:],
                                     hT[k][:, NFREE * t:NFREE * (t + 1)],
                                     start=(k == 0), stop=(k == KI - 1))
            for t in range(NT):
                nc.vector.tensor_scalar_mul(
                    r2T[:, NFREE * t:NFREE * (t + 1)], rps2[t][:], SCALING)

            # ---- phase D: out[T, H] = hidden @ W_dn + lora ----
            # wdnb[k] resident bf16: pure matmul stream, no DMA/cast here.
            # Per stationary hT chunk, both H-halves issue back-to-back.
            nout = 0
            for grp in range(4):
                pos = [[ps_mm.tile([P, NFREE], F32, tag="mm",
                                   name=f"po{grp}_{jj}_{h}")
                        for h in range(NH)] for jj in range(2)]
                for k in range(KI):
                    for jj in range(2):
                        j = 2 * grp + jj
                        for h in range(NH):
                            hs = slice(NFREE * h, NFREE * (h + 1))
                            nc.tensor.matmul(pos[jj][h][:],
                                             hT[k][:, P * j:P * (j + 1)],
                                             wdnb[k][:, hs],
                                             start=(k == 0), stop=False)
                for jj in range(2):
                    j = 2 * grp + jj
                    for h in range(NH):
                        hs = slice(NFREE * h, NFREE * (h + 1))
                        nc.tensor.matmul(pos[jj][h][:], r2T[:, P * j:P * (j + 1)],
                                         bdnT[:, hs], start=False, stop=True)
                        ot = out_pool.tile([P, NFREE], F32, tag="outs")
                        if nout % 2 == 0:
                            nc.scalar.activation(ot[:], pos[jj][h][:],
                                                 mybir.ActivationFunctionType.Copy)
                        else:
                            nc.vector.tensor_copy(ot[:], pos[jj][h][:])
                        nout += 1
                        nc.sync.dma_start(out_d[P * j:P * (j + 1), hs], ot[:])

    nc.finalize()
    return nc


_NC_CACHE = None


def _get_nc():
    global _NC_CACHE
    if _NC_CACHE is None:
        _NC_CACHE = build_kernel()
    return _NC_CACHE


def _run(hidden_states, gate_up_proj, down_proj,
         lora_A_gu, lora_B_gu, lora_A_dn, lora_B_dn, **spmd_kwargs):
    f32 = np.float32
    hidden_states = np.ascontiguousarray(hidden_states, dtype=f32)
    gate_up_proj = np.ascontiguousarray(gate_up_proj, dtype=f32)
    down_proj = np.ascontiguousarray(down_proj, dtype=f32)
    lora_A_gu = np.ascontiguousarray(lora_A_gu, dtype=f32)
    lora_B_gu = np.ascontiguousarray(lora_B_gu, dtype=f32)
    lora_A_dn = np.ascontiguousarray(lora_A_dn, dtype=f32)
    lora_B_dn = np.ascontiguousarray(lora_B_dn, dtype=f32)

    nc = _get_nc()
    in_maps = []
    for e in range(E):
        in_maps.append({
            "x": hidden_states[T * e:T * (e + 1), :],
            "w_gu": gate_up_proj[e],
            "w_dn": down_proj[e],
            "a_gu": lora_A_gu,
            "b_gu": lora_B_gu,
            "a_dn": lora_A_dn,
            "b_dn": lora_B_dn,
        })
    res = run_bass_kernel_spmd(nc, in_maps, core_ids=list(range(E)),
                               **spmd_kwargs)
    out = np.concatenate([res.results[e]["out"] for e in range(E)], axis=0)
    return out, res


def kernel(hidden_states, gate_up_proj, down_proj,
           lora_A_gu, lora_B_gu, lora_A_dn, lora_B_dn):
    out, _ = _run(hidden_states, gate_up_proj, down_proj,
                  lora_A_gu, lora_B_gu, lora_A_dn, lora_B_dn)
    return out
